# revision 3
# baseline (speedup 1.0000x reference)
import os
import sys
import types

import numpy as np

sys.path.insert(0, "/opt/trn_rl_repo")


def _ensure_ntff_hook():
    # The axon boot registers an NTFF-profile hook via antenv.axon_hooks,
    # but this image's antenv package lacks that module, so tracing would
    # silently degrade. Recreate the module and install the ctypes hook.
    try:
        from antenv import axon_hooks  # noqa: F401

        return
    except ImportError:
        pass
    try:
        import antenv
        from trn_agent_boot.trn_boot import _ntff_profile_via_ctypes
    except ImportError:
        return
    mod = types.ModuleType("antenv.axon_hooks")
    _h = [None]
    mod.set_axon_ntff_profile_hook = lambda h: _h.__setitem__(0, h)
    mod.get_axon_ntff_profile_hook = lambda: _h[0]
    sys.modules["antenv.axon_hooks"] = mod
    antenv.axon_hooks = mod
    try:
        hook = _ntff_profile_via_ctypes("/opt/axon/libaxon_pjrt.so")
        if hook is not None:
            mod.set_axon_ntff_profile_hook(hook)
    except Exception:
        pass

from contextlib import ExitStack

from concourse import bass, mybir, tile
from concourse.bass_utils import run_bass_kernel_spmd
from concourse.masks import make_identity

F32 = mybir.dt.float32
U32 = mybir.dt.uint32
AF = mybir.ActivationFunctionType

B, N, S = 8, 8192, 2048
D1, D2, Cin, C1, C2 = 128, 256, 384, 256, 128
P = 128
NT = N // P
TOT = float(B * N)
NN_EPS = 1e-8
BN_EPS = 1e-5
N_CORES = 8

last = {}


def _build_nc():
    nc = bass.Bass()

    dl_pack = nc.declare_dram_parameter("dl_pack", [69, 2816], F32, isOutput=False)
    rhs_rep = nc.declare_dram_parameter("rhs_rep", [69, S], F32, isOutput=False)
    p2t = nc.declare_dram_parameter("p2t", [S, D2], F32, isOutput=False)
    points1 = nc.declare_dram_parameter("points1", [D1, N], F32, isOutput=False)
    w1T = nc.declare_dram_parameter("w1T", [Cin, C1], F32, isOutput=False)
    w2T = nc.declare_dram_parameter("w2T", [C1, C2], F32, isOutput=False)
    b1r = nc.declare_dram_parameter("b1r", [P, 2], F32, isOutput=False)
    g1r = nc.declare_dram_parameter("g1r", [P, 2], F32, isOutput=False)
    be1r = nc.declare_dram_parameter("be1r", [P, 2], F32, isOutput=False)
    b2r = nc.declare_dram_parameter("b2r", [P, 1], F32, isOutput=False)
    g2r = nc.declare_dram_parameter("g2r", [P, 1], F32, isOutput=False)
    be2r = nc.declare_dram_parameter("be2r", [P, 1], F32, isOutput=False)
    out = nc.declare_dram_parameter("out", [C2, N], F32, isOutput=True)

    with tile.TileContext(nc) as tc, ExitStack() as ctx:
        consts = ctx.enter_context(tc.tile_pool(name="consts", bufs=1))
        # p1d: DMA-landing tiles, only read by the Pool copy hop. bufs=8
        # matches the 8 HW-DGE queues so buffer-reuse WAW lands on the same
        # queue semaphore as the own-queue wait (DMA structs allow 2 waits).
        p1d_pool = ctx.enter_context(tc.tile_pool(name="p1d", bufs=8))
        score_pool = ctx.enter_context(tc.tile_pool(name="score", bufs=2))
        topk_pool = ctx.enter_context(tc.tile_pool(name="topk", bufs=4))
        wt_pool = ctx.enter_context(tc.tile_pool(name="wt", bufs=10))
        gath_pool = ctx.enter_context(tc.tile_pool(name="gath", bufs=6))
        interp_pool = ctx.enter_context(tc.tile_pool(name="interp", bufs=4))
        xT_pool = ctx.enter_context(tc.tile_pool(name="xT", bufs=8))
        scratch_pool = ctx.enter_context(tc.tile_pool(name="scratch", bufs=2))
        outc_pool = ctx.enter_context(tc.tile_pool(name="outc", bufs=2))
        psum_d = ctx.enter_context(tc.tile_pool(name="psum_d", bufs=2, space="PSUM"))
        psum_s = ctx.enter_context(tc.tile_pool(name="psum_s", bufs=2, space="PSUM"))
        dram = ctx.enter_context(tc.tile_pool(name="dram", bufs=4, space="DRAM"))

        # ---- constants ----
        # Matmul operands are staged through an in-place Pool-engine copy so
        # PE waits collapse onto one compute semaphore (HW-DGE queue fan-out
        # otherwise exceeds the Matmult struct's sync-wait slots).
        rhs_sb = consts.tile((69, S), F32)
        nc.sync.dma_start(rhs_sb[:], rhs_rep[:])
        nc.gpsimd.tensor_copy(rhs_sb[:], rhs_sb[:])
        dl_sb = consts.tile((69, 2816), F32)
        nc.sync.dma_start(dl_sb[:], dl_pack[:])
        nc.gpsimd.tensor_copy(dl_sb[:], dl_sb[:])
        w1c = []
        for kc in range(3):
            wt = consts.tile((P, C1), F32, name=f"w1c{kc}")
            nc.sync.dma_start(wt[:], w1T[kc * P : (kc + 1) * P, :])
            nc.gpsimd.tensor_copy(wt[:], wt[:])
            w1c.append(wt)
        w2c = []
        for kc in range(2):
            wt = consts.tile((P, C2), F32, name=f"w2c{kc}")
            nc.sync.dma_start(wt[:], w2T[kc * P : (kc + 1) * P, :])
            nc.gpsimd.tensor_copy(wt[:], wt[:])
            w2c.append(wt)
        b1_sb = consts.tile((P, 2), F32)
        nc.sync.dma_start(b1_sb[:], b1r[:])
        g1_sb = consts.tile((P, 2), F32)
        nc.sync.dma_start(g1_sb[:], g1r[:])
        be1_sb = consts.tile((P, 2), F32)
        nc.sync.dma_start(be1_sb[:], be1r[:])
        b2_sb = consts.tile((P, 1), F32)
        nc.sync.dma_start(b2_sb[:], b2r[:])
        g2_sb = consts.tile((P, 1), F32)
        nc.sync.dma_start(g2_sb[:], g2r[:])
        be2_sb = consts.tile((P, 1), F32)
        nc.sync.dma_start(be2_sb[:], be2r[:])
        ident = consts.tile((P, P), F32)
        make_identity(nc, ident[:])
        eps_sb = consts.tile((P, 1), F32)
        nc.vector.memset(eps_sb[:], BN_EPS)

        # ---- persistent activations / stats ----
        y1h = [consts.tile((P, N), F32, name=f"y1h{o}") for o in range(2)]
        y2 = consts.tile((P, N), F32)
        sums1 = [consts.tile((P, NT), F32, name=f"sums1_{o}") for o in range(2)]
        sq1 = [consts.tile((P, NT), F32, name=f"sq1_{o}") for o in range(2)]
        sums2 = consts.tile((P, NT), F32)
        sq2 = consts.tile((P, NT), F32)

        # ---- Phase A: distances, top-3, gather, interp, conv1 (2-stage sw pipeline) ----
        stage = [None] * NT  # stage1 outputs consumed by stage2

        def stage1(t):
            n0 = t * P
            jb = t // 3
            base = 32 * (t % 3)
            lt = dl_sb[base : base + 5, jb * P : (jb + 1) * P]
            p1d = p1d_pool.tile((P, P), F32)
            nc.sync.dma_start(p1d[:], points1[:, n0 : n0 + P], single_packet=True)
            p1 = xT_pool.tile((P, P), F32)
            nc.gpsimd.tensor_copy(p1[:], p1d[:])

            score = score_pool.tile((P, S), F32)
            for c in range(2):
                ps = psum_d.tile((P, 1024), F32)
                for h in range(2):
                    nc.tensor.matmul(
                        ps[:, h * 512 : (h + 1) * 512],
                        lhsT=lt,
                        rhs=rhs_sb[
                            base : base + 5,
                            c * 1024 + h * 512 : c * 1024 + (h + 1) * 512,
                        ],
                        start=True,
                        stop=True,
                    )
                nc.scalar.copy(score[:, c * 1024 : (c + 1) * 1024], ps[:])

            maxv = topk_pool.tile((P, 8), F32)
            nc.vector.max(maxv[:], score[:])
            idx = topk_pool.tile((P, 8), U32)
            nc.vector.max_index(idx[:], maxv[:], score[:])

            # dist_k = -score_k ; recip = 1/(dist+eps); invs = 1/sum(recip)
            dist3 = wt_pool.tile((P, 3), F32)
            nc.scalar.activation(dist3[:], maxv[:, 0:3], AF.Copy, bias=NN_EPS, scale=-1.0)
            recipv = wt_pool.tile((P, 3), F32)
            nc.vector.reciprocal(recipv[:], dist3[:])
            rsum = wt_pool.tile((P, 1), F32)
            nc.vector.reduce_sum(rsum[:], recipv[:], axis=mybir.AxisListType.X)
            invs = wt_pool.tile((P, 1), F32)
            nc.vector.reciprocal(invs[:], rsum[:])

            gs = []
            for k in range(3):
                g = gath_pool.tile((P, D2), F32, name=f"g{k}")
                nc.gpsimd.indirect_dma_start(
                    out=g[:],
                    out_offset=None,
                    in_=p2t[:],
                    in_offset=bass.IndirectOffsetOnAxis(ap=idx[:, k : k + 1], axis=0),
                )
                gs.append(g)
            return (p1, recipv, invs, gs)

        def stage2(t, st):
            n0 = t * P
            p1, recipv, invs, gs = st
            # scale gathered features by recip_k (in place), then sum and normalize
            for k in range(3):
                nc.scalar.activation(gs[k][:], gs[k][:], AF.Copy, scale=recipv[:, k : k + 1])
            acc = interp_pool.tile((P, D2), F32)
            nc.vector.tensor_add(acc[:], gs[0][:], gs[1][:])
            nc.vector.tensor_add(acc[:], acc[:], gs[2][:])
            nc.scalar.activation(acc[:], acc[:], AF.Copy, scale=invs[:])

            itT = []
            for h in range(2):
                tp = psum_s.tile((P, P), F32)
                nc.tensor.transpose(tp[:], acc[:, h * P : (h + 1) * P], ident[:])
                it = xT_pool.tile((P, P), F32)
                nc.scalar.copy(it[:], tp[:])
                itT.append(it)

            rhs3 = [p1, itT[0], itT[1]]
            for o in range(2):
                yps = psum_s.tile((P, P), F32)
                for kc in range(3):
                    nc.tensor.matmul(
                        yps[:],
                        lhsT=w1c[kc][:, o * P : (o + 1) * P],
                        rhs=rhs3[kc][:],
                        start=(kc == 0),
                        stop=(kc == 2),
                    )
                nc.scalar.activation(
                    y1h[o][:, n0 : n0 + P],
                    yps[:],
                    AF.Identity,
                    bias=b1_sb[:, o : o + 1],
                    accum_out=sums1[o][:, t : t + 1],
                )
                sc = scratch_pool.tile((P, P), F32)
                nc.scalar.activation(
                    sc[:],
                    y1h[o][:, n0 : n0 + P],
                    AF.Square,
                    accum_out=sq1[o][:, t : t + 1],
                )

        for t in range(NT + 1):
            if t < NT:
                stage[t] = stage1(t)
            if t >= 1:
                stage2(t - 1, stage[t - 1])
                stage[t - 1] = None

        # ---- BN1 stats AllReduce ----
        stats1 = consts.tile((P, 4), F32)
        nc.vector.reduce_sum(stats1[:, 0:1], sums1[0][:], axis=mybir.AxisListType.X)
        nc.vector.reduce_sum(stats1[:, 1:2], sums1[1][:], axis=mybir.AxisListType.X)
        nc.vector.reduce_sum(stats1[:, 2:3], sq1[0][:], axis=mybir.AxisListType.X)
        nc.vector.reduce_sum(stats1[:, 3:4], sq1[1][:], axis=mybir.AxisListType.X)
        st1_in = dram.tile((P, 4), F32)
        st1_out = dram.tile((P, 4), F32)
        nc.gpsimd.dma_start(st1_in[:], stats1[:])
        nc.gpsimd.collective_compute(
            "AllReduce",
            mybir.AluOpType.add,
            replica_groups=[list(range(N_CORES))],
            ins=[st1_in.opt()],
            outs=[st1_out.opt()],
        )
        ared1 = consts.tile((P, 4), F32)
        nc.gpsimd.dma_start(ared1[:], st1_out[:])

        # scale s1 = gamma/sqrt(var+eps), shift t1 = beta - mean*s1
        def bn_params(ared, nch, g_sb, be_sb):
            m = consts.tile((P, nch), F32)
            nc.scalar.activation(m[:], ared[:, 0:nch], AF.Copy, scale=1.0 / TOT)
            ex2 = consts.tile((P, nch), F32)
            nc.scalar.activation(ex2[:], ared[:, nch : 2 * nch], AF.Copy, scale=1.0 / TOT)
            msq = consts.tile((P, nch), F32)
            nc.scalar.activation(msq[:], m[:], AF.Square)
            var = consts.tile((P, nch), F32)
            nc.vector.tensor_sub(var[:], ex2[:], msq[:])
            sd = consts.tile((P, nch), F32)
            nc.scalar.activation(sd[:], var[:], AF.Sqrt, bias=eps_sb[:])
            rs = consts.tile((P, nch), F32)
            nc.vector.reciprocal(rs[:], sd[:])
            s = consts.tile((P, nch), F32)
            nc.vector.tensor_mul(s[:], rs[:], g_sb[:])
            ms = consts.tile((P, nch), F32)
            nc.vector.tensor_mul(ms[:], m[:], s[:])
            tt = consts.tile((P, nch), F32)
            nc.vector.tensor_sub(tt[:], be_sb[:], ms[:])
            return s, tt

        s1, t1 = bn_params(ared1, 2, g1_sb, be1_sb)

        # ---- Phase B: normalize+relu y1, conv2, stats ----
        for t in range(NT):
            n0 = t * P
            xn = []
            for o in range(2):
                x = xT_pool.tile((P, P), F32)
                nc.scalar.activation(
                    x[:],
                    y1h[o][:, n0 : n0 + P],
                    AF.Relu,
                    bias=t1[:, o : o + 1],
                    scale=s1[:, o : o + 1],
                )
                xn.append(x)
            yps = psum_s.tile((P, P), F32)
            for kc in range(2):
                nc.tensor.matmul(
                    yps[:],
                    lhsT=w2c[kc][:],
                    rhs=xn[kc][:],
                    start=(kc == 0),
                    stop=(kc == 1),
                )
            nc.scalar.activation(
                y2[:, n0 : n0 + P],
                yps[:],
                AF.Identity,
                bias=b2_sb[:, 0:1],
                accum_out=sums2[:, t : t + 1],
            )
            sc = scratch_pool.tile((P, P), F32)
            nc.scalar.activation(
                sc[:], y2[:, n0 : n0 + P], AF.Square, accum_out=sq2[:, t : t + 1]
            )

        # ---- BN2 stats AllReduce ----
        stats2 = consts.tile((P, 2), F32)
        nc.vector.reduce_sum(stats2[:, 0:1], sums2[:], axis=mybir.AxisListType.X)
        nc.vector.reduce_sum(stats2[:, 1:2], sq2[:], axis=mybir.AxisListType.X)
        st2_in = dram.tile((P, 2), F32)
        st2_out = dram.tile((P, 2), F32)
        nc.gpsimd.dma_start(st2_in[:], stats2[:])
        nc.gpsimd.collective_compute(
            "AllReduce",
            mybir.AluOpType.add,
            replica_groups=[list(range(N_CORES))],
            ins=[st2_in.opt()],
            outs=[st2_out.opt()],
        )
        ared2 = consts.tile((P, 2), F32)
        nc.gpsimd.dma_start(ared2[:], st2_out[:])

        s2, t2 = bn_params(ared2, 1, g2_sb, be2_sb)

        # ---- Phase C: normalize+relu y2 -> out ----
        CW = 512
        for c in range(N // CW):
            oc = outc_pool.tile((P, CW), F32)
            nc.scalar.activation(
                oc[:],
                y2[:, c * CW : (c + 1) * CW],
                AF.Relu,
                bias=t2[:, 0:1],
                scale=s2[:, 0:1],
            )
            nc.sync.dma_start(out[:, c * CW : (c + 1) * CW], oc[:])

    import bass_rust

    # Walrus instruction structs hold a single sync wait; this pass splits
    # multi-wait instructions by inserting EventSemaphore (2-wait) preludes.
    bass_rust.generate_event_semaphores(nc)
    return nc


def kernel(**inputs):
    xyz1 = np.ascontiguousarray(inputs["xyz1"], dtype=np.float32)
    xyz2 = np.ascontiguousarray(inputs["xyz2"], dtype=np.float32)
    points1 = np.ascontiguousarray(inputs["points1"], dtype=np.float32)
    points2 = np.ascontiguousarray(inputs["points2"], dtype=np.float32)
    w1 = np.ascontiguousarray(inputs["w1"], dtype=np.float32)
    b1 = np.ascontiguousarray(inputs["b1"], dtype=np.float32)
    gamma1 = np.ascontiguousarray(inputs["gamma1"], dtype=np.float32)
    beta1 = np.ascontiguousarray(inputs["beta1"], dtype=np.float32)
    w2 = np.ascontiguousarray(inputs["w2"], dtype=np.float32)
    b2 = np.ascontiguousarray(inputs["b2"], dtype=np.float32)
    gamma2 = np.ascontiguousarray(inputs["gamma2"], dtype=np.float32)
    beta2 = np.ascontiguousarray(inputs["beta2"], dtype=np.float32)

    w1T = np.ascontiguousarray(w1.T)
    w2T = np.ascontiguousarray(w2.T)
    b1r = np.ascontiguousarray(b1.reshape(2, P).T)
    g1r = np.ascontiguousarray(gamma1.reshape(2, P).T)
    be1r = np.ascontiguousarray(beta1.reshape(2, P).T)
    b2r = np.ascontiguousarray(b2.reshape(P, 1))
    g2r = np.ascontiguousarray(gamma2.reshape(P, 1))
    be2r = np.ascontiguousarray(beta2.reshape(P, 1))

    in_maps = []
    for b in range(N_CORES):
        x1 = xyz1[b]  # [3, N]
        x2 = xyz2[b]  # [3, S]
        x1s = x1 * x1
        n1 = (x1s[0] + x1s[1]) + x1s[2]  # fp32, matches jnp sum order
        x2s = x2 * x2
        n2 = (x2s[0] + x2s[1]) + x2s[2]
        dist_lhsT = np.empty((5, N), np.float32)
        dist_lhsT[0:3] = 2.0 * x1
        dist_lhsT[3] = n1
        dist_lhsT[4] = -1.0
        dist_rhs = np.empty((5, S), np.float32)
        dist_rhs[0:3] = x2
        dist_rhs[3] = -1.0
        dist_rhs[4] = n2
        dl_pack = np.zeros((69, 2816), np.float32)
        for t in range(NT):
            jb = t // 3
            base = 32 * (t % 3)
            dl_pack[base : base + 5, jb * P : (jb + 1) * P] = dist_lhsT[
                :, t * P : (t + 1) * P
            ]
        rhs_rep = np.zeros((69, S), np.float32)
        for base in (0, 32, 64):
            rhs_rep[base : base + 5] = dist_rhs
        in_maps.append(
            {
                "dl_pack": dl_pack,
                "rhs_rep": rhs_rep,
                "p2t": np.ascontiguousarray(points2[b].T),
                "points1": points1[b],
                "w1T": w1T,
                "w2T": w2T,
                "b1r": b1r,
                "g1r": g1r,
                "be1r": be1r,
                "b2r": b2r,
                "g2r": g2r,
                "be2r": be2r,
            }
        )

    nc = _build_nc()
    trace = os.environ.get("KERNEL_TRACE", "1") == "1"
    if trace:
        _ensure_ntff_hook()
    res = run_bass_kernel_spmd(nc, in_maps, list(range(N_CORES)), trace=trace)
    last["exec_time_ns"] = res.exec_time_ns
    last["profile_json"] = res.profile_json
    out = np.stack([res.results[b]["out"] for b in range(N_CORES)], axis=0)
    return out.astype(np.float32)



# revision 4
# speedup vs baseline: 1.0210x; 1.0210x over previous
import os
import sys
import types

import numpy as np

sys.path.insert(0, "/opt/trn_rl_repo")

from contextlib import ExitStack

import ml_dtypes

from concourse import bass, mybir, tile
from concourse.bass_utils import run_bass_kernel_spmd
from concourse.masks import make_identity

F32 = mybir.dt.float32
BF16 = mybir.dt.bfloat16
U32 = mybir.dt.uint32
AF = mybir.ActivationFunctionType
ALU = mybir.AluOpType

B, N, S = 8, 8192, 2048
D1, D2, Cin, C1, C2 = 128, 256, 384, 256, 128
P = 128
NT = N // P
TOT = float(B * N)
NN_EPS = 1e-8
BN_EPS = 1e-5
N_CORES = 8
BATCH = 8  # tiles per recip-chain batch

last = {}


def _ensure_ntff_hook():
    try:
        from antenv import axon_hooks  # noqa: F401

        return
    except ImportError:
        pass
    try:
        import antenv
        from trn_agent_boot.trn_boot import _ntff_profile_via_ctypes
    except ImportError:
        return
    mod = types.ModuleType("antenv.axon_hooks")
    _h = [None]
    mod.set_axon_ntff_profile_hook = lambda h: _h.__setitem__(0, h)
    mod.get_axon_ntff_profile_hook = lambda: _h[0]
    sys.modules["antenv.axon_hooks"] = mod
    antenv.axon_hooks = mod
    try:
        hook = _ntff_profile_via_ctypes("/opt/axon/libaxon_pjrt.so")
        if hook is not None:
            mod.set_axon_ntff_profile_hook(hook)
    except Exception:
        pass


def _build_nc():
    nc = bass.Bass()

    dl_pack = nc.declare_dram_parameter("dl_pack", [5, N], F32, isOutput=False)
    rhs_rep = nc.declare_dram_parameter("rhs_rep", [5, S], F32, isOutput=False)
    p2t = nc.declare_dram_parameter("p2t", [S, D2], BF16, isOutput=False)
    points1 = nc.declare_dram_parameter("points1", [D1, N // 2], U32, isOutput=False)
    w1T = nc.declare_dram_parameter("w1T", [Cin, C1 // 2], U32, isOutput=False)
    w2T = nc.declare_dram_parameter("w2T", [C1, C2 // 2], U32, isOutput=False)
    b1r = nc.declare_dram_parameter("b1r", [P, 2], F32, isOutput=False)
    g1r = nc.declare_dram_parameter("g1r", [P, 2], F32, isOutput=False)
    be1r = nc.declare_dram_parameter("be1r", [P, 2], F32, isOutput=False)
    b2r = nc.declare_dram_parameter("b2r", [P, 1], F32, isOutput=False)
    g2r = nc.declare_dram_parameter("g2r", [P, 1], F32, isOutput=False)
    be2r = nc.declare_dram_parameter("be2r", [P, 1], F32, isOutput=False)
    out = nc.declare_dram_parameter("out", [C2, N], F32, isOutput=True)

    with tile.TileContext(nc) as tc, ExitStack() as ctx:
        consts = ctx.enter_context(tc.tile_pool(name="consts", bufs=1))
        score_pool = ctx.enter_context(tc.tile_pool(name="score", bufs=4))
        mv_pool = ctx.enter_context(tc.tile_pool(name="mv", bufs=3))
        idx_pool = ctx.enter_context(tc.tile_pool(name="idx", bufs=10))
        wt_pool = ctx.enter_context(tc.tile_pool(name="wt", bufs=2))
        gath_pool = ctx.enter_context(tc.tile_pool(name="gath", bufs=20))
        acc_pool = ctx.enter_context(tc.tile_pool(name="acc", bufs=6))
        itT_pool = ctx.enter_context(tc.tile_pool(name="itT", bufs=8))
        scr_pool = ctx.enter_context(tc.tile_pool(name="scr", bufs=1))
        xn_pool = ctx.enter_context(tc.tile_pool(name="xn", bufs=4))
        outc_pool = ctx.enter_context(tc.tile_pool(name="outc", bufs=3))
        psum_d = ctx.enter_context(tc.tile_pool(name="psum_d", bufs=3, space="PSUM"))
        psum_tc = ctx.enter_context(tc.tile_pool(name="psum_tc", bufs=2, space="PSUM"))
        dram = ctx.enter_context(tc.tile_pool(name="dram", bufs=4, space="DRAM"))

        # ---- constants (staged via in-place compute copy for single-sem PE waits;
        # bf16 params ship as u32-packed pairs so the DMA runs at 4-byte-elem rate) ----
        rhs_sb = consts.tile((101, S), F32, name="rhs_sb")
        dl_sb = consts.tile((101, N), F32, name="dl_sb")
        for base in (0, 32, 64, 96):
            nc.sync.dma_start(rhs_sb[base : base + 5, :], rhs_rep[:])
        for base in (0, 32, 64, 96):
            nc.sync.dma_start(dl_sb[base : base + 5, :], dl_pack[:])
        w1c = []
        for kc in range(3):
            t = consts.tile((P, C1), BF16, name=f"w1c{kc}")
            nc.sync.dma_start(t[:].bitcast(U32), w1T[kc * P : (kc + 1) * P, :])
            nc.vector.tensor_copy(t[:], t[:])
            w1c.append(t)
        w2c = []
        for kc in range(2):
            t = consts.tile((P, C2), BF16, name=f"w2c{kc}")
            nc.sync.dma_start(t[:].bitcast(U32), w2T[kc * P : (kc + 1) * P, :])
            nc.vector.tensor_copy(t[:], t[:])
            w2c.append(t)
        p1_sb = consts.tile((D1, N), BF16, name="p1_sb")
        nc.sync.dma_start(p1_sb[:].bitcast(U32), points1[:])
        nc.scalar.copy(p1_sb[:], p1_sb[:])
        b1_sb = consts.tile((P, 2), F32)
        nc.sync.dma_start(b1_sb[:], b1r[:])
        g1_sb = consts.tile((P, 2), F32)
        nc.sync.dma_start(g1_sb[:], g1r[:])
        be1_sb = consts.tile((P, 2), F32)
        nc.sync.dma_start(be1_sb[:], be1r[:])
        b2_sb = consts.tile((P, 1), F32)
        nc.sync.dma_start(b2_sb[:], b2r[:])
        g2_sb = consts.tile((P, 1), F32)
        nc.sync.dma_start(g2_sb[:], g2r[:])
        be2_sb = consts.tile((P, 1), F32)
        nc.sync.dma_start(be2_sb[:], be2r[:])
        ident = consts.tile((P, P), F32)
        make_identity(nc, ident[:])
        eps_sb = consts.tile((P, 1), F32)
        nc.vector.memset(eps_sb[:], BN_EPS)

        # ---- persistent activations / stats ----
        y1h = [consts.tile((P, N), BF16, name=f"y1h{o}") for o in range(2)]
        y2 = consts.tile((P, N), BF16)

        # ---- Phase A ----
        def stage1(t, mvj, tt):
            n0 = t * P
            score = score_pool.tile((P, S), F32)
            # 4-way row-group-concurrent fp32 distance matmuls: the tile's
            # lhsT is replicated at partition bases 0/32/64/96 and the four
            # 512-col chunks stream concurrently into four PSUM banks.
            for c in range(2):
                ps = psum_d.tile((P, 1024), F32)
                for h in range(2):
                    q = c * 2 + h
                    base = 32 * q
                    nc.tensor.matmul(
                        ps[:, h * 512 : (h + 1) * 512],
                        lhsT=dl_sb[base : base + 5, n0 : n0 + P],
                        rhs=rhs_sb[base : base + 5, q * 512 : (q + 1) * 512],
                        start=True,
                        stop=True,
                        tile_position=(base, 0),
                    )
                nc.scalar.copy(score[:, c * 1024 : (c + 1) * 1024], ps[:])

            nc.vector.max(mvj[:, tt, :], score[:])
            idxt = idx_pool.tile((P, 8), U32)
            nc.vector.max_index(idxt[:], mvj[:, tt, :], score[:])

            g = gath_pool.tile((P, 3, D2), BF16)
            for k in range(3):
                nc.gpsimd.indirect_dma_start(
                    out=g[:, k, :],
                    out_offset=None,
                    in_=p2t[:],
                    in_offset=bass.IndirectOffsetOnAxis(ap=idxt[:, k : k + 1], axis=0),
                )
            return g

        def chain(mvj):
            # batched reciprocal-weight chain over BATCH tiles
            wtb = wt_pool.tile((P, BATCH, 3), F32, name="wtb")
            nc.vector.tensor_scalar(
                out=wtb[:],
                in0=mvj[:, :, 0:3],
                scalar1=-1.0,
                scalar2=NN_EPS,
                op0=ALU.mult,
                op1=ALU.add,
            )
            recipb = wt_pool.tile((P, BATCH, 3), F32, name="recipb")
            nc.vector.reciprocal(recipb[:], wtb[:])
            rsumb = wt_pool.tile((P, BATCH), F32, name="rsumb")
            nc.vector.reduce_sum(rsumb[:], recipb[:], axis=mybir.AxisListType.X)
            invsb = wt_pool.tile((P, BATCH), F32, name="invsb")
            nc.vector.reciprocal(invsb[:], rsumb[:])
            invs3 = wt_pool.tile((P, BATCH, 3), F32, name="invs3")
            for k in range(3):
                nc.vector.tensor_copy(invs3[:, :, k], invsb[:])
            cb = wt_pool.tile((P, BATCH, 3), F32, name="cb")
            nc.vector.tensor_mul(cb[:], recipb[:], invs3[:])
            return cb

        def stage2(t, tt, g, cb):
            n0 = t * P
            acc = acc_pool.tile((P, D2), F32)
            m1 = acc_pool.tile((P, D2), BF16, name="m1")
            m2 = acc_pool.tile((P, D2), BF16, name="m2")
            nc.scalar.activation(acc[:], g[:, 0, :], AF.Copy, scale=cb[:, tt, 0:1])
            nc.vector.tensor_scalar_mul(m1[:], g[:, 1, :], cb[:, tt, 1:2])
            nc.vector.tensor_scalar_mul(m2[:], g[:, 2, :], cb[:, tt, 2:3])
            nc.gpsimd.tensor_add(acc[:], acc[:], m1[:])
            nc.gpsimd.tensor_add(acc[:], acc[:], m2[:])

            pst = psum_tc.tile((P, 2 * P), F32, name="pstc")
            for h in range(2):
                nc.tensor.transpose(
                    pst[:, h * P : (h + 1) * P], acc[:, h * P : (h + 1) * P], ident[:]
                )
            itT = itT_pool.tile((P, 2 * P), BF16)
            nc.scalar.copy(itT[:], pst[:])

            rhs3 = [p1_sb[:, n0 : n0 + P], itT[:, 0:P], itT[:, P : 2 * P]]
            psc = psum_tc.tile((P, 2 * P), F32, name="pstc")
            for o in range(2):
                for kc in range(3):
                    nc.tensor.matmul(
                        psc[:, o * P : (o + 1) * P],
                        lhsT=w1c[kc][:, o * P : (o + 1) * P],
                        rhs=rhs3[kc][:] if kc == 0 else rhs3[kc],
                        start=(kc == 0),
                        stop=(kc == 2),
                    )
            for o in range(2):
                nc.scalar.activation(
                    y1h[o][:, n0 : n0 + P],
                    psc[:, o * P : (o + 1) * P],
                    AF.Identity,
                    bias=b1_sb[:, o : o + 1],
                )

        part1 = consts.tile((P, 4), F32)
        part2 = consts.tile((P, 2), F32)

        def _emit_bn1_first_half():
            # first-half BN1 stats while the second half computes
            for o in range(2):
                nc.scalar.activation(
                    y2[:, 0 : N // 2],
                    y1h[o][:, 0 : N // 2],
                    AF.Copy,
                    accum_out=part1[:, o : o + 1],
                )
                nc.scalar.activation(
                    y2[:, 0 : N // 2],
                    y1h[o][:, 0 : N // 2],
                    AF.Square,
                    accum_out=part1[:, 2 + o : 3 + o],
                )

        gs = {}
        mvs = {}
        cbs = {}
        for step in range(NT + BATCH):
            if step >= BATCH:
                tp = step - BATCH
                j2, tt2 = divmod(tp, BATCH)
                stage2(tp, tt2, gs.pop(tp), cbs[j2])
                if tp == NT // 2 - 1:
                    _emit_bn1_first_half()
            if step < NT:
                j, tt = divmod(step, BATCH)
                if tt == 0:
                    mvs[j] = mv_pool.tile((P, BATCH, 8), F32, name="mvj")
                gs[step] = stage1(step, mvs[j], tt)
                if tt == BATCH - 1:
                    cbs[j] = chain(mvs[j])

        # ---- BN1 stats: second-half passes + combine with first half ----
        stats1 = consts.tile((P, 4), F32)
        for o in range(2):
            nc.scalar.activation(
                y2[:, N // 2 : N],
                y1h[o][:, N // 2 : N],
                AF.Copy,
                accum_out=stats1[:, o : o + 1],
            )
            nc.scalar.activation(
                y2[:, N // 2 : N],
                y1h[o][:, N // 2 : N],
                AF.Square,
                accum_out=stats1[:, 2 + o : 3 + o],
            )
        nc.vector.tensor_add(stats1[:], stats1[:], part1[:])
        st1_in = dram.tile((P, 4), F32)
        st1_out = dram.tile((P, 4), F32)
        nc.gpsimd.dma_start(st1_in[:], stats1[:])
        nc.gpsimd.collective_compute(
            "AllReduce",
            ALU.add,
            replica_groups=[list(range(N_CORES))],
            ins=[st1_in.opt()],
            outs=[st1_out.opt()],
        )
        ared1 = consts.tile((P, 4), F32)
        nc.gpsimd.dma_start(ared1[:], st1_out[:])

        def bn_params(ared, nch, g_sb, be_sb):
            m = consts.tile((P, nch), F32)
            nc.scalar.activation(m[:], ared[:, 0:nch], AF.Copy, scale=1.0 / TOT)
            ex2 = consts.tile((P, nch), F32)
            nc.scalar.activation(ex2[:], ared[:, nch : 2 * nch], AF.Copy, scale=1.0 / TOT)
            msq = consts.tile((P, nch), F32)
            nc.scalar.activation(msq[:], m[:], AF.Square)
            var = consts.tile((P, nch), F32)
            nc.vector.tensor_sub(var[:], ex2[:], msq[:])
            sd = consts.tile((P, nch), F32)
            nc.scalar.activation(sd[:], var[:], AF.Sqrt, bias=eps_sb[:])
            rs = consts.tile((P, nch), F32)
            nc.vector.reciprocal(rs[:], sd[:])
            s = consts.tile((P, nch), F32)
            nc.vector.tensor_mul(s[:], rs[:], g_sb[:])
            ms = consts.tile((P, nch), F32)
            nc.vector.tensor_mul(ms[:], m[:], s[:])
            tt_ = consts.tile((P, nch), F32)
            nc.vector.tensor_sub(tt_[:], be_sb[:], ms[:])
            return s, tt_

        s1, t1 = bn_params(ared1, 2, g1_sb, be1_sb)

        # ---- Phase B: normalize+relu y1 (bf16), conv2, stats ----
        for t in range(NT):
            n0 = t * P
            xn = xn_pool.tile((P, 2 * P), BF16)
            # o=0 on ACT
            nc.scalar.activation(
                xn[:, 0:P],
                y1h[0][:, n0 : n0 + P],
                AF.Relu,
                bias=t1[:, 0:1],
                scale=s1[:, 0:1],
            )
            # o=1 on DVE (affine then relu via max(0))
            nc.vector.tensor_scalar(
                out=xn[:, P : 2 * P],
                in0=y1h[1][:, n0 : n0 + P],
                scalar1=s1[:, 1:2],
                scalar2=t1[:, 1:2],
                op0=ALU.mult,
                op1=ALU.add,
            )
            nc.vector.tensor_scalar_max(xn[:, P : 2 * P], xn[:, P : 2 * P], 0.0)

            psc = psum_tc.tile((P, 2 * P), F32, name="pstc")
            for kc in range(2):
                nc.tensor.matmul(
                    psc[:, 0:P],
                    lhsT=w2c[kc][:],
                    rhs=xn[:, kc * P : (kc + 1) * P],
                    start=(kc == 0),
                    stop=(kc == 1),
                )
            nc.scalar.activation(
                y2[:, n0 : n0 + P],
                psc[:, 0:P],
                AF.Identity,
                bias=b2_sb[:, 0:1],
            )
            if t == NT // 2 - 1:
                nc.scalar.activation(
                    y1h[0][:, 0 : N // 2],
                    y2[:, 0 : N // 2],
                    AF.Copy,
                    accum_out=part2[:, 0:1],
                )
                nc.scalar.activation(
                    y1h[0][:, 0 : N // 2],
                    y2[:, 0 : N // 2],
                    AF.Square,
                    accum_out=part2[:, 1:2],
                )

        # ---- BN2 stats: second-half passes + combine ----
        stats2 = consts.tile((P, 2), F32)
        nc.scalar.activation(
            y1h[0][:, N // 2 : N], y2[:, N // 2 : N], AF.Copy, accum_out=stats2[:, 0:1]
        )
        nc.scalar.activation(
            y1h[0][:, N // 2 : N],
            y2[:, N // 2 : N],
            AF.Square,
            accum_out=stats2[:, 1:2],
        )
        nc.vector.tensor_add(stats2[:], stats2[:], part2[:])
        st2_in = dram.tile((P, 2), F32)
        st2_out = dram.tile((P, 2), F32)
        nc.gpsimd.dma_start(st2_in[:], stats2[:])
        nc.gpsimd.collective_compute(
            "AllReduce",
            ALU.add,
            replica_groups=[list(range(N_CORES))],
            ins=[st2_in.opt()],
            outs=[st2_out.opt()],
        )
        ared2 = consts.tile((P, 2), F32)
        nc.gpsimd.dma_start(ared2[:], st2_out[:])

        s2, t2 = bn_params(ared2, 1, g2_sb, be2_sb)

        # ---- Phase C: normalize+relu y2 -> out ----
        CW = 1024
        for c in range(N // CW):
            oc = outc_pool.tile((P, CW), F32)
            nc.scalar.activation(
                oc[:],
                y2[:, c * CW : (c + 1) * CW],
                AF.Relu,
                bias=t2[:, 0:1],
                scale=s2[:, 0:1],
            )
            nc.sync.dma_start(out[:, c * CW : (c + 1) * CW], oc[:])

    import bass_rust

    bass_rust.generate_event_semaphores(nc)
    return nc


def kernel(**inputs):
    xyz1 = np.ascontiguousarray(inputs["xyz1"], dtype=np.float32)
    xyz2 = np.ascontiguousarray(inputs["xyz2"], dtype=np.float32)
    points1 = np.ascontiguousarray(inputs["points1"], dtype=np.float32)
    points2 = np.ascontiguousarray(inputs["points2"], dtype=np.float32)
    w1 = np.ascontiguousarray(inputs["w1"], dtype=np.float32)
    b1 = np.ascontiguousarray(inputs["b1"], dtype=np.float32)
    gamma1 = np.ascontiguousarray(inputs["gamma1"], dtype=np.float32)
    beta1 = np.ascontiguousarray(inputs["beta1"], dtype=np.float32)
    w2 = np.ascontiguousarray(inputs["w2"], dtype=np.float32)
    b2 = np.ascontiguousarray(inputs["b2"], dtype=np.float32)
    gamma2 = np.ascontiguousarray(inputs["gamma2"], dtype=np.float32)
    beta2 = np.ascontiguousarray(inputs["beta2"], dtype=np.float32)

    w1T = np.ascontiguousarray(w1.T).astype(ml_dtypes.bfloat16).view(np.uint32)
    w2T = np.ascontiguousarray(w2.T).astype(ml_dtypes.bfloat16).view(np.uint32)
    b1r = np.ascontiguousarray(b1.reshape(2, P).T)
    g1r = np.ascontiguousarray(gamma1.reshape(2, P).T)
    be1r = np.ascontiguousarray(beta1.reshape(2, P).T)
    b2r = np.ascontiguousarray(b2.reshape(P, 1))
    g2r = np.ascontiguousarray(gamma2.reshape(P, 1))
    be2r = np.ascontiguousarray(beta2.reshape(P, 1))

    in_maps = []
    for b in range(N_CORES):
        x1 = xyz1[b]
        x2 = xyz2[b]
        x1s = x1 * x1
        n1 = (x1s[0] + x1s[1]) + x1s[2]
        x2s = x2 * x2
        n2 = (x2s[0] + x2s[1]) + x2s[2]
        dist_lhsT = np.empty((5, N), np.float32)
        dist_lhsT[0:3] = 2.0 * x1
        dist_lhsT[3] = n1
        dist_lhsT[4] = -1.0
        dist_rhs = np.empty((5, S), np.float32)
        dist_rhs[0:3] = x2
        dist_rhs[3] = -1.0
        dist_rhs[4] = n2
        dl_pack = dist_lhsT
        rhs_rep = dist_rhs
        in_maps.append(
            {
                "b1r": b1r,
                "g1r": g1r,
                "be1r": be1r,
                "b2r": b2r,
                "g2r": g2r,
                "be2r": be2r,
                "w1T": w1T,
                "w2T": w2T,
                "rhs_rep": rhs_rep,
                "dl_pack": dl_pack,
                "points1": np.ascontiguousarray(
                    points1[b].astype(ml_dtypes.bfloat16)
                ).view(np.uint32),
                "p2t": np.ascontiguousarray(points2[b].T).astype(ml_dtypes.bfloat16),
            }
        )

    nc = _build_nc()
    trace = os.environ.get("KERNEL_TRACE", "1") == "1"
    if trace:
        _ensure_ntff_hook()
    res = run_bass_kernel_spmd(nc, in_maps, list(range(N_CORES)), trace=trace)
    last["exec_time_ns"] = res.exec_time_ns
    last["profile_json"] = res.profile_json
    out = np.stack([res.results[b]["out"] for b in range(N_CORES)], axis=0)
    return out.astype(np.float32)
